# revision 1
# baseline (speedup 1.0000x reference)
"""DenseGATConv-style GNN message passing kernel for Trainium2 (Bass/Tile).

Math (per graph b):
    e      = w_edge[edge_attr[b]]            # [N, N] gather from 4-entry table
    adj_w  = adj[b] * e                      # weighted adjacency
    agg    = adj_w @ x[b]                    # [N, C]
    out[b] = agg @ W_rel + b_rel + x[b] @ W_root

Key tricks:
  * The 4-entry gather w_edge[a], a in {0,1,2,3}, equals the cubic polynomial
    through the 4 points, evaluated in factored form
        p(a)/c3 = (a - r) * ((a + h)^2 + v2)
    computed with one ScalarE Square activation + two fused
    scalar_tensor_tensor ops; c3 is folded into W_rel on the host.
  * b_rel is folded into the W_rel matmul as a 65th contraction row against a
    constant ones-row appended to agg^T.
  * The aggregation runs in transposed layout (out^T = Wrel^T@aggT + ...),
    with adj_w transposed on the PE in 128x128 blocks, 8 blocks batched per
    PSUM->SBUF copy.  The whole output stage runs per half-graph (512 rows)
    so the tail latency overlaps the next half's compute.

Sharding: data-parallel over batch B=16 across 8 cores (2 graphs/core);
weights replicated.
"""

import sys
from contextlib import ExitStack

sys.path.insert(0, "/opt/trn_rl_repo")

import numpy as np

_B, _N, _C = 16, 1024, 64
_NCORES = 8
_G = _B // _NCORES  # graphs per core
_P = 128
_NT = _N // _P  # 128-row tiles per graph

# Module-level knobs (test.py may flip these before calling kernel()).
TRACE = False
SQUARE_ENGINE = "act_sq"  # "act_sq" (ScalarE Square) | "dve_stt" (VectorE)
# "float32": exact, PE-bound (~4 cyc/row).  "float32r": TF32-class matmul
# precision (~1.5e-4 rel) but ~2x faster PE for transposes + aggregation.
MM_DTYPE = "float32r"
FINAL_DTYPE = "float32"  # dtype of the small output-transform matmuls
LAST_RESULTS = None  # BassKernelResults of the most recent run (for test.py)

_BUILD_CACHE = {}


def _poly_coeffs(w_edge):
    """Cubic through (k, w_edge[k]) for k=0..3, float64. Returns c0..c3."""
    w = np.asarray(w_edge, dtype=np.float64).reshape(4)
    V = np.vander(np.arange(4.0), 4, increasing=True)
    c = np.linalg.solve(V, w)
    return c  # [c0, c1, c2, c3]


def _chain_params(w_edge):
    """Pick the elementwise chain and host-folded scale from w_edge values.

    Returns (mode, params, lead) where `lead` multiplies W_rel on the host and
    the device computes adj_w/lead.
    """
    c0, c1, c2, c3 = _poly_coeffs(w_edge)
    scale = max(np.max(np.abs(np.asarray(w_edge, dtype=np.float64))), 1e-30)
    tol = 1e-7 * scale
    if abs(c3) > tol:
        # monic cubic a^3 + A a^2 + B a + C = (a - r)(a^2 + p a + q)
        A, Bc, Cc = c2 / c3, c1 / c3, c0 / c3
        roots = np.roots([1.0, A, Bc, Cc])
        r = float(np.real(roots[np.argmin(np.abs(np.imag(roots)))]))
        p = A + r
        q = Bc + p * r
        return "cubic", dict(r=r, p=p, q=q, h=p / 2.0, v2=q - p * p / 4.0), c3
    if abs(c2) > tol:
        p2, q2 = c1 / c2, c0 / c2
        return "quad", dict(p=p2, q=q2, h=p2 / 2.0, v2=q2 - p2 * p2 / 4.0), c2
    if abs(c1) > tol:
        return "linear", dict(r=-c0 / c1), c1
    return "const", dict(), c0


def _emit_square(nc, OP, AF, s_out, ea_ap, params, square_engine, pools):
    """s_out <- quadratic-part tensor; returns the constant to add to it."""
    if square_engine == "act_sq":
        nc.scalar.activation(
            s_out, ea_ap, AF.Square, bias=pools["hbias_sb"][:, 0:1], scale=1.0
        )
        return float(params["v2"])
    nc.vector.scalar_tensor_tensor(
        s_out, ea_ap, float(-params["p"]), ea_ap, OP.subtract, OP.mult
    )
    return float(params["q"])


def _emit_elementwise(
    nc, OP, AF, pools, ea_t, adj_t, mode, params, square_engine, s_pre=None
):
    """Emit adj_w/lead for one [128, N] tile slice pair; returns the aw tile."""
    sp, qtp, awp = pools["sp"], pools["qtp"], pools["awp"]
    mmdt = pools["mmdt"]
    f32 = pools["f32"]

    def square_ap():
        if s_pre is not None:
            return s_pre, float(params["v2"])
        s_t = sp.tile([_P, _N], f32)
        k = _emit_square(nc, OP, AF, s_t[:], ea_t, params, square_engine, pools)
        return s_t[:], k

    if mode == "cubic":
        qt_t = qtp.tile([_P, _N], f32)
        nc.vector.scalar_tensor_tensor(
            qt_t[:], ea_t, float(params["r"]), adj_t, OP.subtract, OP.mult
        )
        s_ap, k_add = square_ap()
        aw_t = awp.tile([_P, _N], mmdt)
        nc.vector.scalar_tensor_tensor(
            aw_t[:], s_ap, k_add, qt_t[:], OP.add, OP.mult
        )
        return aw_t
    if mode == "quad":
        s_ap, k_add = square_ap()
        aw_t = awp.tile([_P, _N], mmdt)
        nc.vector.scalar_tensor_tensor(
            aw_t[:], s_ap, k_add, adj_t, OP.add, OP.mult
        )
        return aw_t
    if mode == "linear":
        aw_t = awp.tile([_P, _N], mmdt)
        nc.vector.scalar_tensor_tensor(
            aw_t[:], ea_t, float(params["r"]), adj_t, OP.subtract, OP.mult
        )
        return aw_t
    aw_t = awp.tile([_P, _N], mmdt)
    nc.vector.tensor_copy(aw_t[:], adj_t)
    return aw_t


def _emit_half(nc, pools, g, half, dram, xs, xs_mm, xT, mode, params, square_engine):
    from concourse import mybir

    OP = mybir.AluOpType
    AF = mybir.ActivationFunctionType
    f32 = pools["f32"]
    mmdt = pools["mmdt"]
    adj_d, ea_d, out_d = dram["adj"], dram["ea"], dram["out"]
    ident = pools["ident"]
    ident_m = pools["ident_m"]
    H = 512

    # DMA + elementwise: 1 MiB chunks of 2 row-tiles; square per chunk,
    # STT chain per [128, 1024] tile
    # x^T columns for this half (root-term operand)
    p_xT = pools["ps_xt"].tile([_C, H], f32, tag="ps_xt")
    for k in range(4):
        jt = 4 * half + k
        nc.tensor.transpose(
            p_xT[:, k * _P : (k + 1) * _P],
            xs[:, jt * _C : (jt + 1) * _C],
            ident[:],
        )
    nc.scalar.copy(out=xT[:, half * H : (half + 1) * H], in_=p_xT[:])

    import contextlib

    aw_tiles = []
    for pair in range(2):
        base = 4 * half + 2 * pair
        hot = g == 0 and half == 0 and pair == 0
        prio = pools["tc"].high_priority() if hot else contextlib.nullcontext()
        with prio:
            ea_t = pools["eap"].tile([_P, 2 * _N], pools["i32"])
            dma_ea = nc.scalar.dma_start(
                out=ea_t[:].rearrange("p (q j) -> p q j", q=2),
                in_=ea_d[g, base * _P : (base + 2) * _P, :].rearrange(
                    "(q p) j -> p q j", p=_P
                ),
            )
            adj_t = pools["adjp"].tile([_P, 2 * _N], f32)
            dma_adj = nc.sync.dma_start(
                out=adj_t[:].rearrange("p (q j) -> p q j", q=2),
                in_=adj_d[g, base * _P : (base + 2) * _P, :].rearrange(
                    "(q p) j -> p q j", p=_P
                ),
            )
        s_t = None
        if square_engine == "act_sq" and mode in ("cubic", "quad"):
            s_t = pools["sp"].tile([_P, 2 * _N], f32)
            nc.scalar.activation(
                s_t[:], ea_t[:], AF.Square,
                bias=pools["hbias_sb"][:, 0:1], scale=1.0,
            )
        for qq in range(2):
            sl1 = slice(qq * _N, (qq + 1) * _N)
            sle = sl1
            aw_t = pools["awp"].tile([_P, _N], mmdt)
            if mode == "cubic":
                qt_t = pools["qtp"].tile([_P, _N], f32)
                nc.vector.scalar_tensor_tensor(
                    qt_t[:], ea_t[:, sle], float(params["r"]), adj_t[:, sl1],
                    OP.subtract, OP.mult,
                )
                if s_t is not None:
                    s_ap, k_add = s_t[:, sle], float(params["v2"])
                else:
                    s_n = pools["sp"].tile([_P, _N], f32, tag="s_dve")
                    nc.vector.scalar_tensor_tensor(
                        s_n[:], ea_t[:, sle], float(-params["p"]), ea_t[:, sle],
                        OP.subtract, OP.mult,
                    )
                    s_ap, k_add = s_n[:], float(params["q"])
                nc.vector.scalar_tensor_tensor(
                    aw_t[:], s_ap, k_add, qt_t[:], OP.add, OP.mult
                )
            elif mode == "quad":
                if s_t is not None:
                    s_ap, k_add = s_t[:, sle], float(params["v2"])
                else:
                    s_n = pools["sp"].tile([_P, _N], f32, tag="s_dve")
                    nc.vector.scalar_tensor_tensor(
                        s_n[:], ea_t[:, sle], float(-params["p"]), ea_t[:, sle],
                        OP.subtract, OP.mult,
                    )
                    s_ap, k_add = s_n[:], float(params["q"])
                nc.vector.scalar_tensor_tensor(
                    aw_t[:], s_ap, k_add, adj_t[:, sl1], OP.add, OP.mult
                )
            elif mode == "linear":
                nc.vector.scalar_tensor_tensor(
                    aw_t[:], ea_t[:, sle], float(params["r"]), adj_t[:, sl1],
                    OP.subtract, OP.mult,
                )
            else:  # const
                nc.vector.tensor_copy(aw_t[:], adj_t[:, sl1])
            aw_tiles.append(aw_t)

    def aw_ap(k, jt):
        return aw_tiles[k][:, jt * _P : (jt + 1) * _P]

    # transpose adj_w blocks (jt-pair batched) + accumulate agg^T over j
    p_aggT = pools["ps_agg"].tile([_C, H], f32, tag="ps_agg")
    for jtp in range(4):
        p_tp = pools["ps_tp"].tile([_P, 2 * H], mmdt, tag="ps_tp")
        for sub in range(2):
            jt = 2 * jtp + sub
            for k in range(4):
                nc.tensor.transpose(
                    p_tp[:, sub * H + k * _P : sub * H + (k + 1) * _P],
                    aw_ap(k, jt),
                    ident_m[:],
                )
        awT = pools["awTp"].tile([_P, 2 * H], mmdt)
        nc.scalar.copy(out=awT[:], in_=p_tp[:])
        for sub in range(2):
            jt = 2 * jtp + sub
            nc.tensor.matmul(
                p_aggT[:],
                lhsT=xs_mm[:, jt * _C : (jt + 1) * _C],
                rhs=awT[:, sub * H : (sub + 1) * H],
                start=(jt == 0),
                stop=(jt == _NT - 1),
            )

    aggT = pools["aggTp"].tile([_C + 1, H], f32)
    nc.vector.memset(aggT[_C : _C + 1, :], 1.0)
    nc.scalar.copy(out=aggT[:_C, :], in_=p_aggT[:])

    # out^T[c', i-half] = [W_rel; b_rel]^T @ [aggT; 1] + W_root^T @ xT
    p_out = pools["ps_out"].tile([_C, H], f32, tag="ps_out")
    nc.tensor.matmul(
        p_out[:], lhsT=pools["wrel_sb"][:], rhs=aggT[:], start=True, stop=False
    )
    nc.tensor.matmul(
        p_out[:], lhsT=pools["wroot_sb"][:],
        rhs=xT[:, half * H : (half + 1) * H],
        start=False, stop=True,
    )
    outT = pools["outTp"].tile([_C, H], f32)
    nc.scalar.copy(out=outT[:], in_=p_out[:])

    # back to natural [i, c] layout and store
    p_on = pools["ps_out"].tile([_P, 4 * _C], f32, tag="ps_out")
    for k in range(4):
        nc.tensor.transpose(
            p_on[:, k * _C : (k + 1) * _C],
            outT[:, k * _P : (k + 1) * _P],
            ident[:_C, :_C],
        )
    out_sb = pools["outp"].tile([_P, 4 * _C], f32)
    nc.scalar.copy(out=out_sb[:], in_=p_on[:])
    nc.sync.dma_start(
        out=out_d[g, half * H : (half + 1) * H, :].rearrange(
            "(t p) c -> p t c", p=_P
        ),
        in_=out_sb[:].rearrange("p (t c) -> p t c", t=4),
    )


def _emit_graph(nc, tc, pools, g, dram, mode, params, square_engine):
    f32 = pools["f32"]
    mmdt = pools["mmdt"]
    x_d = dram["x"]

    # x in aggregation layout: xs[p, t*C+c] = x[t*128+p, c]
    xs = pools["xsp"].tile([_P, _NT * _C], f32)
    nc.sync.dma_start(
        out=xs[:].rearrange("p (t c) -> p t c", t=_NT),
        in_=x_d[g, :, :].rearrange("(t p) c -> p t c", p=_P),
    )
    if mmdt is f32:
        xs_mm = xs
    else:
        xs_mm = pools["xsp"].tile([_P, _NT * _C], mmdt, tag="xs_mm")
        nc.vector.tensor_copy(xs_mm[:], xs[:])
    xT = pools["xTp"].tile([_C, _N], f32)

    for half in range(2):
        _emit_half(
            nc, pools, g, half, dram, xs, xs_mm, xT, mode, params, square_engine
        )


def _build_module(mode, params, square_engine, mm_dtype, final_dtype):
    import concourse.bass as bass  # noqa: F401
    from concourse import bacc, mybir
    from concourse.tile import TileContext

    f32 = mybir.dt.float32
    i32 = mybir.dt.int32
    mmdt = getattr(mybir.dt, mm_dtype)
    fdt = getattr(mybir.dt, final_dtype)
    assert fdt is f32, "FINAL_DTYPE other than float32 not wired up"

    nc = bacc.Bacc(
        "TRN2", target_bir_lowering=False, debug=False, num_devices=_NCORES
    )

    dram = {
        "x": nc.dram_tensor("x", [_G, _N, _C], f32, kind="ExternalInput"),
        "adj": nc.dram_tensor("adj", [_G, _N, _N], f32, kind="ExternalInput"),
        "ea": nc.dram_tensor("ea", [_G, _N, _N], i32, kind="ExternalInput"),
        "wrel": nc.dram_tensor("wrel", [_C + 1, _C], f32, kind="ExternalInput"),
        "wroot": nc.dram_tensor("wroot", [_C, _C], f32, kind="ExternalInput"),
        "ident": nc.dram_tensor("ident", [_P, _P], f32, kind="ExternalInput"),
        "out": nc.dram_tensor("out", [_G, _N, _C], f32, kind="ExternalOutput"),
    }

    pool_specs = [
        ("consts", 1, None),
        ("adjp", 4, None),
        ("eap", 4, None),
        ("sp", 3, None),
        ("qtp", 3, None),
        ("awp", 6, None),
        ("awTp", 3, None),
        ("xsp", 2, None),
        ("xTp", 1, None),
        ("aggTp", 2, None),
        ("outTp", 2, None),
        ("outp", 2, None),
        ("ps_tp", 2, "PSUM"),
        ("ps_agg", 2, "PSUM"),
        ("ps_xt", 1, "PSUM"),
        ("ps_out", 1, "PSUM"),
    ]

    with TileContext(nc) as tc, ExitStack() as ctx:
        pools = {"f32": f32, "i32": i32, "mmdt": mmdt, "fdt": fdt, "tc": tc}
        for name, bufs, space in pool_specs:
            kw = {"space": space} if space else {}
            pools[name] = ctx.enter_context(tc.tile_pool(name=name, bufs=bufs, **kw))

        ident = pools["consts"].tile([_P, _P], f32, tag="ident")
        nc.sync.dma_start(out=ident[:], in_=dram["ident"][:, :])
        pools["ident"] = ident
        if mm_dtype == "float32":
            pools["ident_m"] = ident
        else:
            ident_m = pools["consts"].tile([_P, _P], mmdt, tag="ident_m")
            nc.vector.tensor_copy(ident_m[:], ident[:])
            pools["ident_m"] = ident_m
        for wname, shape in (("wrel", [_C + 1, _C]), ("wroot", [_C, _C])):
            t = pools["consts"].tile(shape, f32, tag=wname)
            nc.sync.dma_start(out=t[:], in_=dram[wname][:, :])
            pools[wname + "_sb"] = t

        if square_engine == "act_sq" and mode in ("cubic", "quad"):
            hb = pools["consts"].tile([_P, 1], f32, tag="hb")
            nc.vector.memset(hb[:], float(params["h"]))
            pools["hbias_sb"] = hb

        for g in range(_G):
            _emit_graph(nc, tc, pools, g, dram, mode, params, square_engine)

    nc.finalize()
    return nc


def _get_module(w_edge, square_engine, mm_dtype, final_dtype="float32"):
    mode, params, lead = _chain_params(w_edge)
    key = (
        mode,
        tuple(sorted((k, round(v, 15)) for k, v in params.items())),
        square_engine,
        mm_dtype,
        final_dtype,
    )
    if key not in _BUILD_CACHE:
        _BUILD_CACHE[key] = _build_module(
            mode, params, square_engine, mm_dtype, final_dtype
        )
    return _BUILD_CACHE[key], lead


def _prep_inputs(x, adj, edge_attr, W_rel, b_rel, W_root, w_edge):
    x = np.ascontiguousarray(np.asarray(x, dtype=np.float32))
    adj = np.ascontiguousarray(np.asarray(adj, dtype=np.float32))
    ea = np.ascontiguousarray(np.asarray(edge_attr, dtype=np.int32).reshape(_B, _N, _N))
    W_rel = np.asarray(W_rel, dtype=np.float64)
    W_root = np.ascontiguousarray(np.asarray(W_root, dtype=np.float32))
    b_rel = np.asarray(b_rel, dtype=np.float32).reshape(1, _C)
    w_edge = np.asarray(w_edge)
    return x, adj, ea, W_rel, b_rel, W_root, w_edge


def kernel(x, adj, edge_attr, W_rel, b_rel, W_root, w_edge):
    global LAST_RESULTS
    from concourse.bass_utils import run_bass_kernel_spmd

    x, adj, ea, W_rel, b_rel, W_root, w_edge = _prep_inputs(
        x, adj, edge_attr, W_rel, b_rel, W_root, w_edge
    )
    nc, lead = _get_module(w_edge, SQUARE_ENGINE, MM_DTYPE, FINAL_DTYPE)
    wrel_eff = np.ascontiguousarray(
        np.concatenate([lead * W_rel, b_rel.astype(np.float64)], axis=0).astype(
            np.float32
        )
    )
    ident = np.eye(_P, dtype=np.float32)

    in_maps = []
    for c in range(_NCORES):
        sl = slice(c * _G, (c + 1) * _G)
        in_maps.append(
            {
                "x": x[sl],
                "adj": adj[sl],
                "ea": ea[sl],
                "wrel": wrel_eff,
                "wroot": W_root,
                "ident": ident,
            }
        )

    res = run_bass_kernel_spmd(nc, in_maps, list(range(_NCORES)), trace=TRACE)
    LAST_RESULTS = res
    out = np.concatenate([res.results[c]["out"] for c in range(_NCORES)], axis=0)
    return out



# revision 21
# speedup vs baseline: 1.9843x; 1.9843x over previous
"""DenseGATConv-style GNN message passing kernel for Trainium2 (Bass/Tile).

Math (per graph b):
    e      = w_edge[edge_attr[b]]            # [N, N] gather from 4-entry table
    adj_w  = adj[b] * e                      # weighted adjacency
    agg    = adj_w @ x[b]                    # [N, C]
    out[b] = agg @ W_rel + b_rel + x[b] @ W_root

Key design points (v2):
  * Inputs are staged TRANSPOSED on the host (adj^T, edge_attr^T) so the
    elementwise chain produces adj_w^T tiles directly in [j-part, i-free]
    layout -- the aggregation matmul contracts over j on the PE with zero
    on-chip transposes (v1 spent ~72us/core on PE transposes).
  * The 4-entry gather w_edge[a], a in {0,1,2,3}, is fit EXACTLY by
        w_edge[a] = alpha * sin(beta*a + gamma) + delta
    (4 unknowns, 4 equation -- solved on host in fp64).  On device this is
    ONE ScalarE Sin activation + ONE VectorE scalar_tensor_tensor:
        s   = Sin(beta*a + gamma)            # ScalarE, uint8 in, fp16 out
        awT = (s + delta/alpha) * adjT       # DVE STT, 2x mode (16-bit)
    alpha is folded into W_rel on the host.
  * Everything on-chip runs in fp16 (adj in [0,1) and |e|<~1.4 are exactly
    representable to 2^-11): halves HBM traffic and hits the DVE 2x perf
    mode; edge_attr ships as uint8 (1B/elem instead of 4).
  * Output transform: out[i,c] = [aggT; xT].T @ [alpha*W_rel; W_root] + b_rel
    as one 128-contraction matmul per 128-row chunk; b_rel enters via a
    K=1 ones-row matmul that initializes the PSUM accumulator.

Sharding: data-parallel over batch B=16 across 8 cores (2 graphs/core);
weights replicated.
"""

import sys
from contextlib import ExitStack

sys.path.insert(0, "/opt/trn_rl_repo")

import numpy as np

_B, _N, _C = 16, 1024, 64
_NCORES = 8
_G = _B // _NCORES  # graphs per core
_P = 128
_NT = _N // _P  # 128-row tiles per graph
_H = 512  # half-graph columns (one PSUM bank of fp32)
_CHUNK = 2  # row-tiles per DMA/elementwise chunk

# Module-level knobs (test.py may flip these before calling kernel()).
TRACE = False
EA_DTYPE = "uint8"  # "uint8" | "float16" (fallback if u8 activation fails)
LAST_RESULTS = None  # BassKernelResults of the most recent run (for test.py)

_BUILD_CACHE = {}


def _poly_coeffs(w_edge):
    """Cubic through (k, w_edge[k]) for k=0..3, float64. Returns c0..c3."""
    w = np.asarray(w_edge, dtype=np.float64).reshape(4)
    V = np.vander(np.arange(4.0), 4, increasing=True)
    return np.linalg.solve(V, w)


def _act_fit(f, w, n_starts=6000, seed=0):
    """Exact 4-point fit w[a] = alpha*f(beta*a+gamma)+delta via random-start
    Gauss-Newton (numpy only).  Returns (beta, gamma, alpha, delta) or None."""
    w = np.asarray(w, dtype=np.float64).reshape(4)
    a4 = np.arange(4.0)
    scale = max(np.max(np.abs(w)), 1e-30)
    rng = np.random.default_rng(seed)
    best = None
    for _ in range(n_starts):
        b = rng.uniform(-4.0, 4.0)
        g = rng.uniform(-8.0, 8.0)
        M = np.stack([f(b * a4 + g), np.ones(4)], axis=1)
        sol, *_ = np.linalg.lstsq(M, w, rcond=None)
        r = M @ sol - w
        v = float(r @ r)
        if best is None or v < best[0]:
            best = (v, b, g, float(sol[0]), float(sol[1]))
    p = np.array(best[1:], dtype=np.float64)
    eps = 1e-6
    for _ in range(200):
        b, g, al, de = p
        r = al * f(b * a4 + g) + de - w
        if np.abs(r).max() < 1e-12 * scale:
            break
        J = np.empty((4, 4))
        for j in range(4):
            q = p.copy()
            q[j] += eps
            J[:, j] = (q[2] * f(q[0] * a4 + q[1]) + q[3] - w - r) / eps
        try:
            step, *_ = np.linalg.lstsq(J, r, rcond=None)
        except np.linalg.LinAlgError:
            return None
        p = p - step
    b, g, al, de = p
    r = al * f(b * a4 + g) + de - w
    if np.abs(r).max() < 1e-9 * scale and abs(al) > 1e-9 * scale:
        return float(b), float(g), float(al), float(de)
    return None


def _fit_chain(w_edge):
    """Pick the device elementwise chain for e = w_edge[a], a in {0..3}.

    Preferred: exact silu fit  e = alpha*silu(beta*a+gamma) + delta
    (one ScalarE activation, unbounded domain, + tensor_scalar(+k) at 4x
    + tensor_tensor(*adjT) at 2x on the DVE).  Then sin (domain limited
    to [-pi,pi] on ScalarE, checked).  Falls back to the factored cubic.

    Returns (mode, params, lead): device computes awT = chain(a) * adjT
    such that true adj_w = lead * awT; `lead` is folded into W_rel.
    """
    w = np.asarray(w_edge, dtype=np.float64).reshape(4)
    v0, v1, v2, v3 = w
    scale = max(np.max(np.abs(w)), 1e-30)

    def silu(x):
        return x / (1.0 + np.exp(-np.clip(x, -60, 60)))

    fit = _act_fit(silu, w)
    if fit is not None and abs(fit[0]) * 3 + abs(fit[1]) < 30.0:
        b, g, al, de = fit
        return "silu", dict(beta=b, gamma=g, k=float(de / al)), al

    # sin fit: recurrence s_{k+1} + s_{k-1} = 2 cos(beta) s_k for s_k = v_k - d
    den = (v0 + v2) + 2.0 * v2 - (v1 + v3) - 2.0 * v1
    if abs(den) > 1e-9 * scale:
        d = ((v0 + v2) * v2 - (v1 + v3) * v1) / den
        if abs(v1 - d) > 1e-9 * scale:
            c = (v0 + v2 - 2.0 * d) / (2.0 * (v1 - d))
            if abs(c) < 1.0 - 1e-7:
                b = float(np.arccos(c))
                sb = np.sin(b)
                P = v0 - d  # alpha*sin(gamma)
                Q = ((v1 - d) - P * c) / sb  # alpha*cos(gamma)
                alpha = float(np.hypot(P, Q))
                g = float(np.arctan2(P, Q))
                args = b * np.arange(4.0) + g
                fitv = alpha * np.sin(args) + d
                if (
                    np.abs(fitv - w).max() < 1e-9 * scale
                    and alpha > 1e-9 * scale
                    and np.abs(args).max() <= np.pi  # ScalarE Sin domain
                ):
                    return (
                        "sin",
                        dict(beta=b, gamma=g, k=float(d / alpha)),
                        alpha,
                    )

    c0, c1, c2, c3 = _poly_coeffs(w)
    tol = 1e-7 * scale
    if abs(c3) > tol:
        # monic cubic a^3+A a^2+B a+C = (a-r)((a+h)^2 + v2)
        A, Bc, Cc = c2 / c3, c1 / c3, c0 / c3
        roots = np.roots([1.0, A, Bc, Cc])
        r = float(np.real(roots[np.argmin(np.abs(np.imag(roots)))]))
        p = A + r
        q = Bc + p * r
        return "cubic", dict(r=r, h=p / 2.0, v2=q - p * p / 4.0), c3
    if abs(c2) > tol:
        p2, q2 = c1 / c2, c0 / c2
        return "quad", dict(h=p2 / 2.0, v2=q2 - p2 * p2 / 4.0), c2
    if abs(c1) > tol:
        return "linear", dict(r=-c0 / c1), c1
    return "const", dict(), c0


def _emit_graph(nc, pools, g, dram, mode, params, ea_is_u8):
    from concourse import mybir

    OP = mybir.AluOpType
    AF = mybir.ActivationFunctionType
    f32 = pools["f32"]
    f16 = pools["f16"]
    adjT_d, eaT_d, x_d, xT_d, out_d = (
        dram["adjT"], dram["eaT"], dram["x"], dram["xT"], dram["out"],
    )

    CW = _CHUNK * _N  # elements per chunk row (free dim)
    n_chunks = _NT // _CHUNK

    bf16 = pools["bf16"]

    # x in lhsT layout: xs[p, t*C+c] = x[t*128+p, c]
    xs = pools["xsp"].tile([_P, _NT * _C], bf16)
    nc.sync.dma_start(
        out=xs[:].rearrange("p (t c) -> p t c", t=_NT),
        in_=x_d[g, :, :].rearrange("(t p) c -> p t c", p=_P),
    )

    # stacked lhsT for the output transform: rows 0:64 <- aggT (later),
    # rows 64:128 <- xT (DMA now)
    stk = []
    for half in range(2):
        t = pools["stkp"].tile([_P, _H], bf16, tag=f"stk{half}")
        nc.scalar.dma_start(
            out=t[_C : 2 * _C, :],
            in_=xT_d[g, :, half * _H : (half + 1) * _H],
        )
        stk.append(t)

    p_agg = [
        pools["ps_agg"].tile(
            [_C, _H], f32, tag=f"ps_agg{half}", name=f"p_agg{half}"
        )
        for half in range(2)
    ]

    import contextlib

    for ch in range(n_chunks):
        hot = g == 0 and ch == 0
        prio = pools["tc"].high_priority() if hot else contextlib.nullcontext()
        with prio:
            eaT_t = pools["eap"].tile(
                [_P, CW], pools["u8"] if ea_is_u8 else f16
            )
            nc.scalar.dma_start(
                out=eaT_t[:].rearrange("p (q j) -> p q j", q=_CHUNK),
                in_=eaT_d[g, ch * _CHUNK * _P : (ch + 1) * _CHUNK * _P, :].rearrange(
                    "(q p) j -> p q j", p=_P
                ),
            )
            adjT_t = pools["adjp"].tile([_P, CW], f16)
            nc.sync.dma_start(
                out=adjT_t[:].rearrange("p (q j) -> p q j", q=_CHUNK),
                in_=adjT_d[g, ch * _CHUNK * _P : (ch + 1) * _CHUNK * _P, :].rearrange(
                    "(q p) j -> p q j", p=_P
                ),
            )

        awT_t = pools["awp"].tile([_P, CW], bf16)
        if mode in ("silu", "sin"):
            s_t = pools["sp"].tile([_P, CW], f16)
            nc.scalar.activation(
                s_t[:], eaT_t[:],
                AF.Silu if mode == "silu" else AF.Sin,
                bias=pools["abias_sb"][:, 0:1], scale=float(params["beta"]),
            )
            # s2 = s + k at 4x, awT = s2 * adjT at 2x (scalar_tensor_tensor
            # has no 2x uop -- always 1x -- so the split is faster)
            s2_t = pools["s2p"].tile([_P, CW], f16)
            nc.vector.tensor_scalar(
                s2_t[:], s_t[:], float(params["k"]), None, OP.add
            )
            nc.vector.tensor_tensor(awT_t[:], s2_t[:], adjT_t[:], OP.mult)
        elif mode == "cubic":
            s_t = pools["sp"].tile([_P, CW], f16)
            nc.scalar.activation(
                s_t[:], eaT_t[:], AF.Square,
                bias=pools["abias_sb"][:, 0:1], scale=1.0,
            )
            qt_t = pools["qtp"].tile([_P, CW], f16)
            nc.vector.scalar_tensor_tensor(
                qt_t[:], eaT_t[:], float(params["r"]), adjT_t[:],
                OP.subtract, OP.mult,
            )
            nc.vector.scalar_tensor_tensor(
                awT_t[:], s_t[:], float(params["v2"]), qt_t[:], OP.add, OP.mult
            )
        elif mode == "quad":
            s_t = pools["sp"].tile([_P, CW], f16)
            nc.scalar.activation(
                s_t[:], eaT_t[:], AF.Square,
                bias=pools["abias_sb"][:, 0:1], scale=1.0,
            )
            nc.vector.scalar_tensor_tensor(
                awT_t[:], s_t[:], float(params["v2"]), adjT_t[:], OP.add, OP.mult
            )
        elif mode == "linear":
            nc.vector.scalar_tensor_tensor(
                awT_t[:], eaT_t[:], float(params["r"]), adjT_t[:],
                OP.subtract, OP.mult,
            )
        else:  # const
            nc.vector.tensor_copy(awT_t[:], adjT_t[:])

        # accumulate agg^T[c, i] += x_jt^T @ awT_jt for both halves
        for sub in range(_CHUNK):
            jt = ch * _CHUNK + sub
            for half in range(2):
                nc.tensor.matmul(
                    p_agg[half][:],
                    lhsT=xs[:, jt * _C : (jt + 1) * _C],
                    rhs=awT_t[:, sub * _N + half * _H : sub * _N + (half + 1) * _H],
                    start=(jt == 0),
                    stop=(jt == _NT - 1),
                )

    # output transform per half
    for half in range(2):
        nc.vector.tensor_copy(stk[half][:_C, :], p_agg[half][:])
        outb = pools["outp"].tile([_P, 4 * _C], f32)
        for q in range(4):
            p_out = pools["ps_out"].tile([_P, _C], f32, tag="ps_out")
            # b_rel via K=1 ones-row matmul (initializes the accumulator)
            nc.tensor.matmul(
                p_out[:],
                lhsT=pools["ones_sb"][:, :],
                rhs=pools["brel_sb"][:, :],
                start=True, stop=False,
            )
            nc.tensor.matmul(
                p_out[:],
                lhsT=stk[half][:, q * _P : (q + 1) * _P],
                rhs=pools["wstack_sb"][:, :],
                start=False, stop=True,
            )
            nc.vector.tensor_copy(outb[:, q * _C : (q + 1) * _C], p_out[:])
        nc.sync.dma_start(
            out=out_d[g, half * _H : (half + 1) * _H, :].rearrange(
                "(q p) c -> p q c", p=_P
            ),
            in_=outb[:].rearrange("p (q c) -> p q c", q=4),
        )


def _build_module(mode, params, ea_is_u8):
    import concourse.bass as bass  # noqa: F401
    from concourse import bacc, mybir
    from concourse.tile import TileContext

    f32 = mybir.dt.float32
    f16 = mybir.dt.float16
    bf16 = mybir.dt.bfloat16
    u8 = mybir.dt.uint8

    nc = bacc.Bacc(
        "TRN2", target_bir_lowering=False, debug=False, num_devices=_NCORES
    )

    dram = {
        "adjT": nc.dram_tensor("adjT", [_G, _N, _N], f16, kind="ExternalInput"),
        "eaT": nc.dram_tensor(
            "eaT", [_G, _N, _N], u8 if ea_is_u8 else f16, kind="ExternalInput"
        ),
        "x": nc.dram_tensor("x", [_G, _N, _C], bf16, kind="ExternalInput"),
        "xT": nc.dram_tensor("xT", [_G, _C, _N], bf16, kind="ExternalInput"),
        "wstack": nc.dram_tensor(
            "wstack", [2 * _C, _C], bf16, kind="ExternalInput"
        ),
        "brel": nc.dram_tensor("brel", [1, _C], bf16, kind="ExternalInput"),
        "out": nc.dram_tensor("out", [_G, _N, _C], f32, kind="ExternalOutput"),
    }

    pool_specs = [
        ("consts", 1, None),
        ("adjp", 4, None),
        ("eap", 4, None),
        ("sp", 3, None),
        ("s2p", 3, None),
        ("qtp", 3, None),
        ("awp", 3, None),
        ("xsp", 2, None),
        ("stkp", 4, None),
        ("outp", 3, None),
        ("ps_agg", 2, "PSUM"),
        ("ps_out", 4, "PSUM"),
    ]

    with TileContext(nc) as tc, ExitStack() as ctx:
        pools = {"f32": f32, "f16": f16, "bf16": bf16, "u8": u8, "tc": tc}
        for name, bufs, space in pool_specs:
            kw = {"space": space} if space else {}
            pools[name] = ctx.enter_context(tc.tile_pool(name=name, bufs=bufs, **kw))

        wstack = pools["consts"].tile([2 * _C, _C], bf16, tag="wstack")
        nc.sync.dma_start(out=wstack[:], in_=dram["wstack"][:, :])
        pools["wstack_sb"] = wstack
        brel = pools["consts"].tile([1, _C], bf16, tag="brel")
        nc.sync.dma_start(out=brel[:], in_=dram["brel"][:, :])
        pools["brel_sb"] = brel
        ones = pools["consts"].tile([1, _P], bf16, tag="ones")
        nc.vector.memset(ones[:], 1.0)
        pools["ones_sb"] = ones

        # per-partition activation bias (gamma for Silu/Sin, h for Square)
        if mode in ("silu", "sin", "cubic", "quad"):
            ab = pools["consts"].tile([_P, 1], f32, tag="abias")
            bias_val = params["gamma"] if mode in ("silu", "sin") else params["h"]
            nc.vector.memset(ab[:], float(bias_val))
            pools["abias_sb"] = ab

        for g in range(_G):
            _emit_graph(nc, pools, g, dram, mode, params, ea_is_u8)

    nc.finalize()
    return nc


def _get_module(w_edge, ea_dtype):
    mode, params, lead = _fit_chain(w_edge)
    ea_is_u8 = ea_dtype == "uint8" and mode in ("silu", "sin", "quad", "const")
    # cubic/linear read ea on the DVE -> needs a float dtype
    key = (
        mode,
        tuple(sorted((k, round(v, 15)) for k, v in params.items())),
        ea_is_u8,
    )
    if key not in _BUILD_CACHE:
        _BUILD_CACHE[key] = _build_module(mode, params, ea_is_u8)
    return _BUILD_CACHE[key], lead, ea_is_u8


def kernel(x, adj, edge_attr, W_rel, b_rel, W_root, w_edge):
    global LAST_RESULTS
    from concourse import mybir
    from concourse.bass_utils import run_bass_kernel_spmd

    f16np = mybir.dt.np(mybir.dt.float16)
    bf16np = mybir.dt.np(mybir.dt.bfloat16)

    x = np.asarray(x, dtype=np.float32)
    adj = np.asarray(adj, dtype=np.float32)
    ea = np.asarray(edge_attr, dtype=np.int32).reshape(_B, _N, _N)
    W_rel = np.asarray(W_rel, dtype=np.float64)
    W_root = np.asarray(W_root, dtype=np.float64)
    b_rel = np.asarray(b_rel, dtype=np.float64).reshape(1, _C)
    w_edge = np.asarray(w_edge)

    nc, lead, ea_is_u8 = _get_module(w_edge, EA_DTYPE)

    adjT = np.ascontiguousarray(adj.transpose(0, 2, 1)).astype(f16np)
    eaT = np.ascontiguousarray(ea.transpose(0, 2, 1))
    eaT = eaT.astype(np.uint8) if ea_is_u8 else eaT.astype(f16np)
    x16 = x.astype(bf16np)
    xT = np.ascontiguousarray(x.transpose(0, 2, 1)).astype(bf16np)
    wstack = np.ascontiguousarray(
        np.concatenate([lead * W_rel, W_root], axis=0)
    ).astype(bf16np)
    brel = np.ascontiguousarray(b_rel).astype(bf16np)

    in_maps = []
    for c in range(_NCORES):
        sl = slice(c * _G, (c + 1) * _G)
        in_maps.append(
            {
                "adjT": adjT[sl],
                "eaT": eaT[sl],
                "x": x16[sl],
                "xT": xT[sl],
                "wstack": wstack,
                "brel": brel,
            }
        )

    res = run_bass_kernel_spmd(nc, in_maps, list(range(_NCORES)), trace=TRACE)
    LAST_RESULTS = res
    out = np.concatenate([res.results[c]["out"] for c in range(_NCORES)], axis=0)
    return out


# revision 26
# speedup vs baseline: 2.0099x; 1.0129x over previous
"""DenseGATConv-style GNN message passing kernel for Trainium2 (Bass/Tile).

Math (per graph b):
    e      = w_edge[edge_attr[b]]            # [N, N] gather from 4-entry table
    adj_w  = adj[b] * e                      # weighted adjacency
    agg    = adj_w @ x[b]                    # [N, C]
    out[b] = agg @ W_rel + b_rel + x[b] @ W_root

Key design points (v2):
  * Inputs are staged TRANSPOSED on the host (adj^T, edge_attr^T) so the
    elementwise chain produces adj_w^T tiles directly in [j-part, i-free]
    layout -- the aggregation matmul contracts over j on the PE with zero
    on-chip transposes (v1 spent ~72us/core on PE transposes).
  * The 4-entry gather w_edge[a], a in {0,1,2,3}, is fit EXACTLY by
        w_edge[a] = alpha * sin(beta*a + gamma) + delta
    (4 unknowns, 4 equation -- solved on host in fp64).  On device this is
    ONE ScalarE Sin activation + ONE VectorE scalar_tensor_tensor:
        s   = Sin(beta*a + gamma)            # ScalarE, uint8 in, fp16 out
        awT = (s + delta/alpha) * adjT       # DVE STT, 2x mode (16-bit)
    alpha is folded into W_rel on the host.
  * Everything on-chip runs in fp16 (adj in [0,1) and |e|<~1.4 are exactly
    representable to 2^-11): halves HBM traffic and hits the DVE 2x perf
    mode; edge_attr ships as uint8 (1B/elem instead of 4).
  * Output transform: out[i,c] = [aggT; xT].T @ [alpha*W_rel; W_root] + b_rel
    as one 128-contraction matmul per 128-row chunk; b_rel enters via a
    K=1 ones-row matmul that initializes the PSUM accumulator.

Sharding: data-parallel over batch B=16 across 8 cores (2 graphs/core);
weights replicated.
"""

import sys
from contextlib import ExitStack

sys.path.insert(0, "/opt/trn_rl_repo")

import numpy as np

_B, _N, _C = 16, 1024, 64
_NCORES = 8
_G = _B // _NCORES  # graphs per core
_P = 128
_NT = _N // _P  # 128-row tiles per graph
_H = 512  # half-graph columns (one PSUM bank of fp32)
_CHUNK = 2  # row-tiles per DMA/elementwise chunk

# Module-level knobs (test.py may flip these before calling kernel()).
TRACE = False
EA_DTYPE = "uint8"  # "uint8" | "float16" (fallback if u8 activation fails)
LAST_RESULTS = None  # BassKernelResults of the most recent run (for test.py)

_BUILD_CACHE = {}


def _poly_coeffs(w_edge):
    """Cubic through (k, w_edge[k]) for k=0..3, float64. Returns c0..c3."""
    w = np.asarray(w_edge, dtype=np.float64).reshape(4)
    V = np.vander(np.arange(4.0), 4, increasing=True)
    return np.linalg.solve(V, w)


def _act_fit(f, w, n_starts=6000, seed=0):
    """Exact 4-point fit w[a] = alpha*f(beta*a+gamma)+delta via random-start
    Gauss-Newton (numpy only).  Returns (beta, gamma, alpha, delta) or None."""
    w = np.asarray(w, dtype=np.float64).reshape(4)
    a4 = np.arange(4.0)
    scale = max(np.max(np.abs(w)), 1e-30)
    rng = np.random.default_rng(seed)
    best = None
    for _ in range(n_starts):
        b = rng.uniform(-4.0, 4.0)
        g = rng.uniform(-8.0, 8.0)
        M = np.stack([f(b * a4 + g), np.ones(4)], axis=1)
        sol, *_ = np.linalg.lstsq(M, w, rcond=None)
        r = M @ sol - w
        v = float(r @ r)
        if best is None or v < best[0]:
            best = (v, b, g, float(sol[0]), float(sol[1]))
    p = np.array(best[1:], dtype=np.float64)
    eps = 1e-6
    for _ in range(200):
        b, g, al, de = p
        r = al * f(b * a4 + g) + de - w
        if np.abs(r).max() < 1e-12 * scale:
            break
        J = np.empty((4, 4))
        for j in range(4):
            q = p.copy()
            q[j] += eps
            J[:, j] = (q[2] * f(q[0] * a4 + q[1]) + q[3] - w - r) / eps
        try:
            step, *_ = np.linalg.lstsq(J, r, rcond=None)
        except np.linalg.LinAlgError:
            return None
        p = p - step
    b, g, al, de = p
    r = al * f(b * a4 + g) + de - w
    if np.abs(r).max() < 1e-9 * scale and abs(al) > 1e-9 * scale:
        return float(b), float(g), float(al), float(de)
    return None


def _fit_chain(w_edge):
    """Pick the device elementwise chain for e = w_edge[a], a in {0..3}.

    Preferred: exact silu fit  e = alpha*silu(beta*a+gamma) + delta
    (one ScalarE activation, unbounded domain, + tensor_scalar(+k) at 4x
    + tensor_tensor(*adjT) at 2x on the DVE).  Then sin (domain limited
    to [-pi,pi] on ScalarE, checked).  Falls back to the factored cubic.

    Returns (mode, params, lead): device computes awT = chain(a) * adjT
    such that true adj_w = lead * awT; `lead` is folded into W_rel.
    """
    w = np.asarray(w_edge, dtype=np.float64).reshape(4)
    v0, v1, v2, v3 = w
    scale = max(np.max(np.abs(w)), 1e-30)

    def silu(x):
        return x / (1.0 + np.exp(-np.clip(x, -60, 60)))

    fit = _act_fit(silu, w)
    if fit is not None and abs(fit[0]) * 3 + abs(fit[1]) < 30.0:
        b, g, al, de = fit
        return "silu", dict(beta=b, gamma=g, k=float(de / al)), al

    # sin fit: recurrence s_{k+1} + s_{k-1} = 2 cos(beta) s_k for s_k = v_k - d
    den = (v0 + v2) + 2.0 * v2 - (v1 + v3) - 2.0 * v1
    if abs(den) > 1e-9 * scale:
        d = ((v0 + v2) * v2 - (v1 + v3) * v1) / den
        if abs(v1 - d) > 1e-9 * scale:
            c = (v0 + v2 - 2.0 * d) / (2.0 * (v1 - d))
            if abs(c) < 1.0 - 1e-7:
                b = float(np.arccos(c))
                sb = np.sin(b)
                P = v0 - d  # alpha*sin(gamma)
                Q = ((v1 - d) - P * c) / sb  # alpha*cos(gamma)
                alpha = float(np.hypot(P, Q))
                g = float(np.arctan2(P, Q))
                args = b * np.arange(4.0) + g
                fitv = alpha * np.sin(args) + d
                if (
                    np.abs(fitv - w).max() < 1e-9 * scale
                    and alpha > 1e-9 * scale
                    and np.abs(args).max() <= np.pi  # ScalarE Sin domain
                ):
                    return (
                        "sin",
                        dict(beta=b, gamma=g, k=float(d / alpha)),
                        alpha,
                    )

    c0, c1, c2, c3 = _poly_coeffs(w)
    tol = 1e-7 * scale
    if abs(c3) > tol:
        # monic cubic a^3+A a^2+B a+C = (a-r)((a+h)^2 + v2)
        A, Bc, Cc = c2 / c3, c1 / c3, c0 / c3
        roots = np.roots([1.0, A, Bc, Cc])
        r = float(np.real(roots[np.argmin(np.abs(np.imag(roots)))]))
        p = A + r
        q = Bc + p * r
        return "cubic", dict(r=r, h=p / 2.0, v2=q - p * p / 4.0), c3
    if abs(c2) > tol:
        p2, q2 = c1 / c2, c0 / c2
        return "quad", dict(h=p2 / 2.0, v2=q2 - p2 * p2 / 4.0), c2
    if abs(c1) > tol:
        return "linear", dict(r=-c0 / c1), c1
    return "const", dict(), c0


def _emit_graph(nc, pools, g, dram, mode, params, ea_is_u8):
    from concourse import mybir

    OP = mybir.AluOpType
    AF = mybir.ActivationFunctionType
    f32 = pools["f32"]
    f16 = pools["f16"]
    adjT_d, eaT_d, x_d, xT_d, out_d = (
        dram["adjT"], dram["eaT"], dram["x"], dram["xT"], dram["out"],
    )

    CW = _CHUNK * _N  # elements per chunk row (free dim)
    n_chunks = _NT // _CHUNK

    bf16 = pools["bf16"]

    # x in lhsT layout (host pre-tiled): xs[p, t*C+c] = x[t*128+p, c]
    xs = pools["xsp"].tile([_P, _NT * _C], bf16)
    nc.scalar.dma_start(out=xs[:], in_=x_d[g, :, :])

    # stacked lhsT for the output transform: rows 0:64 <- aggT (later),
    # rows 64:128 <- xT (DMA now)
    stk = []
    for half in range(2):
        t = pools["stkp"].tile([_P, _H], bf16, tag=f"stk{half}")
        nc.scalar.dma_start(
            out=t[_C : 2 * _C, :],
            in_=xT_d[g, :, half * _H : (half + 1) * _H],
        )
        stk.append(t)

    p_agg = [
        pools["ps_agg"].tile(
            [_C, _H], f32, tag=f"ps_agg{half}", name=f"p_agg{half}"
        )
        for half in range(2)
    ]

    import contextlib

    for ch in range(n_chunks):
        hot = g == 0 and ch == 0
        prio = pools["tc"].high_priority() if hot else contextlib.nullcontext()
        with prio:
            eaT_t = pools["eap"].tile(
                [_P, CW], pools["u8"] if ea_is_u8 else f16
            )
            nc.scalar.dma_start(
                out=eaT_t[:], in_=eaT_d[g, :, ch * CW : (ch + 1) * CW]
            )
            adjT_t = pools["adjp"].tile([_P, CW], f16)
            nc.sync.dma_start(
                out=adjT_t[:], in_=adjT_d[g, :, ch * CW : (ch + 1) * CW]
            )

        awT_t = pools["awp"].tile([_P, CW], bf16)
        if mode in ("silu", "sin"):
            s_t = pools["sp"].tile([_P, CW], f16)
            nc.scalar.activation(
                s_t[:], eaT_t[:],
                AF.Silu if mode == "silu" else AF.Sin,
                bias=pools["abias_sb"][:, 0:1], scale=float(params["beta"]),
            )
            # s2 = s + k at 4x, awT = s2 * adjT at 2x (scalar_tensor_tensor
            # has no 2x uop -- always 1x -- so the split is faster)
            s2_t = pools["s2p"].tile([_P, CW], f16)
            nc.vector.tensor_scalar(
                s2_t[:], s_t[:], float(params["k"]), None, OP.add
            )
            nc.vector.tensor_tensor(awT_t[:], s2_t[:], adjT_t[:], OP.mult)
        elif mode == "cubic":
            s_t = pools["sp"].tile([_P, CW], f16)
            nc.scalar.activation(
                s_t[:], eaT_t[:], AF.Square,
                bias=pools["abias_sb"][:, 0:1], scale=1.0,
            )
            qt_t = pools["qtp"].tile([_P, CW], f16)
            nc.vector.scalar_tensor_tensor(
                qt_t[:], eaT_t[:], float(params["r"]), adjT_t[:],
                OP.subtract, OP.mult,
            )
            nc.vector.scalar_tensor_tensor(
                awT_t[:], s_t[:], float(params["v2"]), qt_t[:], OP.add, OP.mult
            )
        elif mode == "quad":
            s_t = pools["sp"].tile([_P, CW], f16)
            nc.scalar.activation(
                s_t[:], eaT_t[:], AF.Square,
                bias=pools["abias_sb"][:, 0:1], scale=1.0,
            )
            nc.vector.scalar_tensor_tensor(
                awT_t[:], s_t[:], float(params["v2"]), adjT_t[:], OP.add, OP.mult
            )
        elif mode == "linear":
            nc.vector.scalar_tensor_tensor(
                awT_t[:], eaT_t[:], float(params["r"]), adjT_t[:],
                OP.subtract, OP.mult,
            )
        else:  # const
            nc.vector.tensor_copy(awT_t[:], adjT_t[:])

        # accumulate agg^T[c, i] += x_jt^T @ awT_jt for both halves
        for sub in range(_CHUNK):
            jt = ch * _CHUNK + sub
            for half in range(2):
                nc.tensor.matmul(
                    p_agg[half][:],
                    lhsT=xs[:, jt * _C : (jt + 1) * _C],
                    rhs=awT_t[:, sub * _N + half * _H : sub * _N + (half + 1) * _H],
                    start=(jt == 0),
                    stop=(jt == _NT - 1),
                )

    # output transform per half
    for half in range(2):
        nc.scalar.copy(out=stk[half][:_C, :], in_=p_agg[half][:])
        outb = pools["outp"].tile([_P, 4 * _C], f32)
        for q in range(4):
            p_out = pools["ps_out"].tile([_P, _C], f32, tag="ps_out")
            nc.tensor.matmul(
                p_out[:],
                lhsT=stk[half][:, q * _P : (q + 1) * _P],
                rhs=pools["wstack_sb"][:, :],
                start=True, stop=True,
            )
            # copy + b_rel add in one pass (brelf is host-pre-broadcast)
            nc.vector.tensor_tensor(
                outb[:, q * _C : (q + 1) * _C], p_out[:],
                pools["brelf_sb"][:, :], OP.add,
            )
        # out dram is host-staged as [G, 2, 128, 4C]: direct 2D store
        nc.sync.dma_start(out=out_d[g, half, :, :], in_=outb[:])


def _build_module(mode, params, ea_is_u8):
    import concourse.bass as bass  # noqa: F401
    from concourse import bacc, mybir
    from concourse.tile import TileContext

    f32 = mybir.dt.float32
    f16 = mybir.dt.float16
    bf16 = mybir.dt.bfloat16
    u8 = mybir.dt.uint8

    nc = bacc.Bacc(
        "TRN2", target_bir_lowering=False, debug=False, num_devices=_NCORES
    )

    # adjT/eaT/x are host pre-tiled to [.., 128 partitions, contiguous free]
    # so every DMA is one fat contiguous descriptor per partition.
    dram = {
        "adjT": nc.dram_tensor(
            "adjT", [_G, _P, _NT * _N], f16, kind="ExternalInput"
        ),
        "eaT": nc.dram_tensor(
            "eaT", [_G, _P, _NT * _N], u8 if ea_is_u8 else f16,
            kind="ExternalInput",
        ),
        "x": nc.dram_tensor("x", [_G, _P, _NT * _C], bf16, kind="ExternalInput"),
        "xT": nc.dram_tensor("xT", [_G, _C, _N], bf16, kind="ExternalInput"),
        "wstack": nc.dram_tensor(
            "wstack", [2 * _C, _C], bf16, kind="ExternalInput"
        ),
        "brelf": nc.dram_tensor("brelf", [_P, _C], bf16, kind="ExternalInput"),
        "out": nc.dram_tensor(
            "out", [_G, 2, _P, 4 * _C], f32, kind="ExternalOutput"
        ),
    }

    pool_specs = [
        ("consts", 1, None),
        ("adjp", 4, None),
        ("eap", 4, None),
        ("sp", 3, None),
        ("s2p", 3, None),
        ("qtp", 3, None),
        ("awp", 3, None),
        ("xsp", 2, None),
        ("stkp", 4, None),
        ("outp", 3, None),
        ("ps_agg", 2, "PSUM"),
        ("ps_out", 4, "PSUM"),
    ]

    with TileContext(nc) as tc, ExitStack() as ctx:
        pools = {"f32": f32, "f16": f16, "bf16": bf16, "u8": u8, "tc": tc}
        for name, bufs, space in pool_specs:
            kw = {"space": space} if space else {}
            pools[name] = ctx.enter_context(tc.tile_pool(name=name, bufs=bufs, **kw))

        wstack = pools["consts"].tile([2 * _C, _C], bf16, tag="wstack")
        nc.sync.dma_start(out=wstack[:], in_=dram["wstack"][:, :])
        pools["wstack_sb"] = wstack
        brelf = pools["consts"].tile([_P, _C], bf16, tag="brelf")
        nc.sync.dma_start(out=brelf[:], in_=dram["brelf"][:, :])
        pools["brelf_sb"] = brelf

        # per-partition activation bias (gamma for Silu/Sin, h for Square)
        if mode in ("silu", "sin", "cubic", "quad"):
            ab = pools["consts"].tile([_P, 1], f32, tag="abias")
            bias_val = params["gamma"] if mode in ("silu", "sin") else params["h"]
            nc.vector.memset(ab[:], float(bias_val))
            pools["abias_sb"] = ab

        for g in range(_G):
            _emit_graph(nc, pools, g, dram, mode, params, ea_is_u8)

    nc.finalize()
    return nc


def _get_module(w_edge, ea_dtype):
    mode, params, lead = _fit_chain(w_edge)
    ea_is_u8 = ea_dtype == "uint8" and mode in ("silu", "sin", "quad", "const")
    # cubic/linear read ea on the DVE -> needs a float dtype
    key = (
        mode,
        tuple(sorted((k, round(v, 15)) for k, v in params.items())),
        ea_is_u8,
    )
    if key not in _BUILD_CACHE:
        _BUILD_CACHE[key] = _build_module(mode, params, ea_is_u8)
    return _BUILD_CACHE[key], lead, ea_is_u8


def kernel(x, adj, edge_attr, W_rel, b_rel, W_root, w_edge):
    global LAST_RESULTS
    from concourse import mybir
    from concourse.bass_utils import run_bass_kernel_spmd

    f16np = mybir.dt.np(mybir.dt.float16)
    bf16np = mybir.dt.np(mybir.dt.bfloat16)

    x = np.asarray(x, dtype=np.float32)
    adj = np.asarray(adj, dtype=np.float32)
    ea = np.asarray(edge_attr, dtype=np.int32).reshape(_B, _N, _N)
    W_rel = np.asarray(W_rel, dtype=np.float64)
    W_root = np.asarray(W_root, dtype=np.float64)
    b_rel = np.asarray(b_rel, dtype=np.float64).reshape(1, _C)
    w_edge = np.asarray(w_edge)

    nc, lead, ea_is_u8 = _get_module(w_edge, EA_DTYPE)

    def tile_rows(a):
        """[B, N, F] -> [B, 128, NT*F]: row j*128+p of graph b lands at
        [b, p, j*F:(j+1)*F] -- one contiguous free-dim line per partition."""
        B, N, F = a.shape
        return np.ascontiguousarray(
            a.reshape(B, _NT, _P, F).transpose(0, 2, 1, 3).reshape(B, _P, _NT * F)
        )

    adjT = tile_rows(np.ascontiguousarray(adj.transpose(0, 2, 1))).astype(f16np)
    eaT = tile_rows(np.ascontiguousarray(ea.transpose(0, 2, 1)))
    eaT = eaT.astype(np.uint8) if ea_is_u8 else eaT.astype(f16np)
    x16 = tile_rows(x).astype(bf16np)
    xT = np.ascontiguousarray(x.transpose(0, 2, 1)).astype(bf16np)
    wstack = np.ascontiguousarray(
        np.concatenate([lead * W_rel, W_root], axis=0)
    ).astype(bf16np)
    brelf = np.ascontiguousarray(np.broadcast_to(b_rel, (_P, _C))).astype(bf16np)

    in_maps = []
    for c in range(_NCORES):
        sl = slice(c * _G, (c + 1) * _G)
        in_maps.append(
            {
                "adjT": adjT[sl],
                "eaT": eaT[sl],
                "x": x16[sl],
                "xT": xT[sl],
                "wstack": wstack,
                "brelf": brelf,
            }
        )

    res = run_bass_kernel_spmd(nc, in_maps, list(range(_NCORES)), trace=TRACE)
    LAST_RESULTS = res
    # out is staged [G, 2, 128, 4*C]: row i = half*512 + q*128 + p
    outs = np.concatenate(
        [np.asarray(res.results[c]["out"]) for c in range(_NCORES)], axis=0
    ).astype(np.float32)
    out = (
        outs.reshape(_B, 2, _P, 4, _C)
        .transpose(0, 1, 3, 2, 4)
        .reshape(_B, _N, _C)
    )
    return np.ascontiguousarray(out)
